# revision 32
# baseline (speedup 1.0000x reference)
"""Trainium2 Bass kernel for nn_AttentionBlock (B=8, C=512, H=W=32, 8 heads).

Sharding: data-parallel over batch — core b computes batch image b end-to-end
(weights replicated to all 8 cores).

Design notes (cost model: matmul time = moving-operand columns only; moving
dtype sets the rate, bf16 = 1 col/cycle at any N):
  P1a: q,k = Wqk^T.T @ x -> (1024, S), channel order arranged on host so each
       128-row tile is one head-PAIR of q or k.  All operands bf16.
  P1b: vT = x.T @ Wv^T -> (S, 512) so attention needs no transposes.
  sc : scoresT[key, q] per (j-tile, head): 2 matmuls of N=512 into a PSUM
       ping-pong slot; ACT exp -> et (bf16, SBUF) at scale 1/8.
  AV : FLIPPED vs the usual orientation — out[q-tile, d] = et-chunk^T @ v
       streams only N=64 v-columns per (head, j, q-chunk): 8x less moving
       data than streaming queries.  Denominators via extra N=1 matmuls
       (rhs = ones column) into a dedicated PSUM bank.
       AV for pair p runs one pair-loop LATE (during pair p+1's score/exp
       rounds) so the PE always has exp-independent work while ACT (the
       64x 1038ns exp stream) paces the attention phase.
  norm: DVE reciprocal of the 16 denominator columns, then one stride-0
       broadcast tensor_tensor per head: evict+normalize PSUM->SBUF bf16.
  T  : PE transposes (bf16 identity moving => 1 cycle/row) turn res'[q, c]
       into res[c, q] for the output projection.
  P4 : y = Wo^T.T @ resT + bo + x, STT on DVE, y DMA per m-tile.

PSUM banks: 0-3 scores ping-pong (2 slots x 4KB), 4-5 AV accumulators
(2 heads x 8 q-tiles x 64), 6 P1/transpose scratch, 7 denominators.
Interleaved PSUM accumulation groups share banks via the 2KB zero-region
rule: only the first matmul touching a bank-epoch uses start=True; other
groups' first writes auto-zero through the pending-zero flag.
"""

import os
import sys

for _p in ("/opt/trn_rl_repo", "/root/.axon_site/_ro/trn_rl_repo"):
    if os.path.isdir(_p) and _p not in sys.path:
        sys.path.insert(0, _p)

from contextlib import ExitStack

import ml_dtypes
import numpy as np

import concourse.bass as bass
import concourse.tile as tile
from concourse import mybir
from concourse.bass_utils import run_bass_kernel_spmd

B, C, H, W = 8, 512, 32, 32
NH, D = 8, 64
S = H * W            # 1024 sequence positions
P = 128              # partitions
KT = C // P          # 4 contraction tiles over channels
NT = S // P          # 8 key/query tiles
NPAIR = NH // 2      # 4 head pairs
NWCOL = 4 * C + P    # wallb cols: 2C qk | C v | C wo | 128 identity
F32 = mybir.dt.float32
BF16 = mybir.dt.bfloat16
AF = mybir.ActivationFunctionType
ALU = mybir.AluOpType

EXP_BUFS = int(os.environ.get("K_EXP_BUFS", "24"))
WARM_BIG = int(os.environ.get("K_WARM_BIG", "8"))
WARM_SMALL = int(os.environ.get("K_WARM_SMALL", "4"))


def _install_drain_split():
    """walrus's CTRL_NO (drain) codegen accepts only a single semaphore wait,
    but Tile's kernel-tail drain aggregates one wait per live proc.  Split
    them across several serial drains (semantically identical: all complete
    before the closing all-engine barrier)."""
    if getattr(tile.TileContext, "_drain_split_installed", False):
        return
    from concourse.vector_clock import ScopedClock

    orig = tile.TileContext._drain_and_barrier

    def patched(self, tick_clock, wait_clock):
        nc = self.nc
        drain_inst = nc.sync.drain()
        wait_clock.add_sem_waits(
            drain_inst.ins, ScopedClock({None: tick_clock.global_clock})
        )
        si = drain_inst.ins.sync_info
        if si is not None and si.on_wait and len(si.on_wait) > 1:
            waits = list(si.on_wait)
            drain_inst.ins.sync_info = mybir.SyncInfo(
                on_wait=[waits[0]], on_update=list(si.on_update or [])
            )
            for w in waits[1:]:
                d2 = nc.sync.drain()
                d2.ins.sync_info = mybir.SyncInfo(on_wait=[w], on_update=[])

        nc.all_engine_barrier()
        assert self.sems is not None
        popped = nc._tile_sem_poison_stack.pop()
        assert popped is self._sem_poison
        nc.clear_and_free_semaphores(list(self.sems.allocated().values()))
        nc.all_engine_barrier()

    tile.TileContext._drain_and_barrier = patched
    tile.TileContext._drain_split_installed = True
    tile.TileContext._drain_and_barrier_orig = orig


def trace_kernel(ctx, tc, nc, xb, wallb, bof, y):
    cst = ctx.enter_context(tc.tile_pool(name="cst", bufs=1))
    qkp = ctx.enter_context(tc.tile_pool(name="qkp", bufs=4))
    expp = ctx.enter_context(tc.tile_pool(name="expp", bufs=EXP_BUFS))
    rdp = ctx.enter_context(tc.tile_pool(name="rdp", bufs=2))
    rqp = ctx.enter_context(tc.tile_pool(name="rqp", bufs=2))
    yp = ctx.enter_context(tc.tile_pool(name="yp", bufs=1))
    # PSUM pools, allocation order = bank order (8 banks total):
    # scp: scores ping-pong 2x[128,1024] (4 banks) + yB accs at the tail;
    # oap: AV accumulators / spare P1 scratch (2 banks); p1p: P1/transpose/yA
    # scratch (1 bank); dnp: denominators (1 bank).
    scp = ctx.enter_context(tc.tile_pool(name="scp", bufs=2, space="PSUM"))
    oap = ctx.enter_context(tc.tile_pool(name="oap", bufs=1, space="PSUM"))
    p1p = ctx.enter_context(tc.tile_pool(name="p1p", bufs=1, space="PSUM"))
    dnp = ctx.enter_context(tc.tile_pool(name="dnp", bufs=1, space="PSUM"))

    xt = cst.tile([P, KT, S], BF16)
    wall = cst.tile([P, KT, NWCOL], BF16)
    wqkt = wall[:, :, 0:2 * C]
    wvt = wall[:, :, 2 * C:3 * C]
    wot = wall[:, :, 3 * C:4 * C]
    ident = wall[:, 0, 4 * C:4 * C + P]          # [128, 128] bf16 identity

    bo_sb = cst.tile([P, KT], F32)
    onesc = cst.tile([P, 1], BF16)
    vta = cst.tile([P, NT, C], BF16)             # v^T tiles, head-major cols
    resT = cst.tile([P, KT, S], BF16)            # res[c, s], k-tile = pair
    scr = cst.tile([1, 256], F32)
    scrp = cst.tile([1, 16], F32)    # Pool-engine carrier scratch
    identb = cst.tile([P, P], BF16)
    warm = cst.tile([P, 640], BF16)
    yax = cst.tile([P, KT, S], F32)              # yA partial (k=0,1) + x
    ybig = yp.tile([P, KT, S], BF16)

    # ---- input DMA, chunked so the first p1a epoch starts early ----
    xr = xb.rearrange("(k p) s -> p k s", p=P)
    wr = wallb.rearrange("(k p) c -> p k c", p=P)
    nc.gpsimd.dma_start(out=wall[:, :, 0:256], in_=wr[:, :, 0:256])
    nc.sync.dma_start(out=xt[:, :, 0:512], in_=xr[:, :, 0:512])
    nc.sync.dma_start(out=xt[:, :, 512:S], in_=xr[:, :, 512:S])
    nc.gpsimd.dma_start(out=wall[:, :, 2 * C:3 * C], in_=wr[:, :, 2 * C:3 * C])
    nc.gpsimd.dma_start(out=wall[:, :, 256:2 * C], in_=wr[:, :, 256:2 * C])
    nc.gpsimd.dma_start(out=wall[:, :, 3 * C:NWCOL], in_=wr[:, :, 3 * C:NWCOL])
    nc.gpsimd.dma_start(out=bo_sb.unsqueeze(2),
                        in_=bof.rearrange("(k p) o -> p k o", p=P))

    nc.vector.memset(onesc[:, :], 1.0)

    scr_i = [0]

    def dve_sync(*aps):
        # DVE wait-carrier: absorb one cross-engine wait per tiny copy.
        # Callers pass 2-D APs (partition x free).
        for ap in aps:
            n = min(ap.free_size(), 16)
            o = (scr_i[0] % 15) * 16
            scr_i[0] += 1
            nc.vector.tensor_copy(scr[0:1, o:o + n], ap[0:1, 0:n])

    def pe_mm(corner, dep):
        # PE wait-carrier: a 1x2 matmul reading `dep` absorbs one cross-
        # engine wait; PE program order subsumes the tick for later matmuls.
        nc.tensor.matmul(
            corner, dep[0:1, 0:1], dep[0:1, 0:2],
            start=True, stop=True, skip_group_check=True,
        )

    # ---------------- scores + exp (2-slot ping-pong, 2 exps/j) ------------
    pending_pe_syncs = []
    ets_hist = []
    pair_ets = [[None] * (2 * NT) for _ in range(NPAIR)]

    def scores_round(pair, j):
        qk = qk_tiles[pair]
        for hh in range(2):
            sc = scp.tile([P, S], F32, tag="sc", name=f"sc{pair}_{j}_{hh}")
            idx = len(ets_hist)
            if idx >= 2:
                pe_mm(sc[0:1, 0:2], ets_hist[idx - 2])
            while pending_pe_syncs:
                pe_mm(sc[0:1, 0:2], pending_pe_syncs.pop())
            for n in range(2):
                nc.tensor.matmul(
                    sc[:, n * 512:(n + 1) * 512],
                    qk[64 * hh:64 * (hh + 1), S + j * P: S + (j + 1) * P],
                    qk[64 * hh:64 * (hh + 1), n * 512:(n + 1) * 512],
                    start=True, stop=True,
                )
            et = expp.tile([P, S], BF16, tag="et", name=f"et{pair}_{j}_{hh}")
            nc.scalar.activation(et[:, :], sc[:, :], AF.Exp,
                                 scale=1.0 / np.sqrt(D))
            ets_hist.append(et)
            pair_ets[pair][2 * j + hh] = et

    # ---------------- P1 epochs on alternating scratch banks ---------------
    qk_tiles = [None] * NPAIR
    use_oap = [False]

    def scratch(name, force=None):
        # alternate between the p1p bank and the (free) oap bank
        if force is None:
            use_oap[0] = not use_oap[0]
            pool = oap if use_oap[0] else p1p
        else:
            pool = force
        return pool.tile([P, 512], F32, tag="oa" if pool is oap else "p1",
                         name=name)

    def p1a_epoch(m, n, first=False, force=None):
        pair, isk = divmod(m, 2)
        if isk == 0 and n == 0:
            qk_tiles[pair] = qkp.tile([P, 2 * S], BF16, tag="qk",
                                      name=f"qk{pair}")
        acc = scratch(f"p1a{m}_{n}", force)
        if first:
            pe_mm(acc[0:1, 0:2], xt[:, 0, 0:2])
            pe_mm(acc[0:1, 0:2], wall[:, 0, 0:2])
        for k in range(KT):
            nc.tensor.matmul(
                acc[:, :],
                wqkt[:, k, m * P:(m + 1) * P],
                xt[:, k, n * 512:(n + 1) * 512],
                start=(k == 0), stop=(k == KT - 1),
            )
        with nc.allow_low_precision(reason="bf16 qk tiles"):
            nc.vector.tensor_copy(
                qk_tiles[pair][:, isk * S + n * 512: isk * S + (n + 1) * 512],
                acc[:, :],
            )

    def p1b_epoch(j):
        acc = scratch(f"p1b{j}")
        for k in range(KT):
            nc.tensor.matmul(
                acc[:, :],
                xt[:, k, j * P:(j + 1) * P],
                wvt[:, k, :],
                start=(k == 0), stop=(k == KT - 1),
            )
        with nc.allow_low_precision(reason="bf16 v tiles"):
            nc.vector.tensor_copy(vta[:, j, :], acc[:, :])

    def ya_group(m, n):
        acc = scratch(f"ya{m}_{n}", force=p1p)
        for k in range(2):
            nc.tensor.matmul(
                acc[:, :],
                wot[:, k, m * P:(m + 1) * P],
                resT[:, k, n * 512:(n + 1) * 512],
                start=(k == 0), stop=(k == 1),
            )
        nc.vector.scalar_tensor_tensor(
            yax[:, m, n * 512:(n + 1) * 512], acc[:, :], 0.0,
            xt[:, m, n * 512:(n + 1) * 512], op0=ALU.add, op1=ALU.add,
        )

    # ---------------- AV + norm + transpose --------------------------------
    av_state = {}

    def av_open(pair):
        av_state["oa"] = oap.tile([P, 1024], F32, tag="oa", name=f"oa{pair}")
        av_state["den"] = dnp.tile([P, 16], F32, tag="den", name=f"den{pair}")

    def av_round(pair, j):
        oa, den = av_state["oa"], av_state["den"]
        for hh in range(2):
            et = pair_ets[pair][2 * j + hh]
            v = vta[:, j, (2 * pair + hh) * D:(2 * pair + hh + 1) * D]
            for t in range(NT):
                nc.tensor.matmul(
                    oa[:, hh * 512 + t * D: hh * 512 + (t + 1) * D],
                    et[:, t * P:(t + 1) * P], v,
                    start=(j == 0 and t == 0), stop=(j == NT - 1),
                    skip_group_check=True,
                )
                nc.tensor.matmul(
                    den[:, hh * NT + t: hh * NT + t + 1],
                    et[:, t * P:(t + 1) * P], onesc[:, :],
                    start=(j == 0 and t == 0 and hh == 0),
                    stop=(j == NT - 1),
                    skip_group_check=True,
                )

    def norm_pair(pair):
        oa, den = av_state["oa"], av_state["den"]
        # sample-read carriers absorb the PE waits (schedule-correct values)
        dve_sync(den[0:1, 0:16])
        for hh in range(2):
            dve_sync(oa[0:1, hh * 512:(hh + 1) * 512].rearrange(
                "p (t d) -> p t d", t=NT)[:, :, 0:1].rearrange(
                "p t d -> p (t d)"))
        rd = rdp.tile([P, 16], F32, tag="rd", name=f"rd{pair}")
        nc.vector.reciprocal(rd[:, :], den[:, :])
        resq = rqp.tile([P, NT, P], BF16, tag="rq", name=f"resq{pair}")
        with nc.allow_low_precision(reason="bf16 res tiles"):
            for hh in range(2):
                nc.vector.tensor_tensor(
                    resq[:, :, hh * D:(hh + 1) * D],
                    oa[:, hh * 512:(hh + 1) * 512].rearrange(
                        "p (t d) -> p t d", t=NT),
                    rd[:, hh * NT:(hh + 1) * NT].unsqueeze(2).broadcast_to(
                        [P, NT, D]),
                    op=ALU.mult,
                )
        return resq

    def transpose_pair(pair, resq):
        tp = p1p.tile([P, NT * P], BF16, tag="p1", name=f"tp{pair}")
        for t in range(NT):
            nc.tensor.transpose(
                tp[:, t * P:(t + 1) * P], resq[:, t, :], identb[:, :])
        nc.vector.tensor_copy(resT[:, pair, :], tp[:, :])

    # ================= schedule =================
    dummy = scp.tile([P, S], F32, tag="sc", name="dummy")
    dve_sync(xt[0:1, 0, 0:8])
    dve_sync(xt[0:1, 0, 512:520])
    # warm-up: spin the PE while the input DMA lands so the p-state ramp
    # completes before real work
    nc.vector.memset(warm[:, :], 0.25)
    for i in range(WARM_BIG):
        nc.tensor.matmul(dummy[:, 0:512], warm[:, 0:128], warm[:, 128:640],
                         start=True, stop=True, skip_group_check=True)
    for i in range(WARM_SMALL):
        nc.tensor.matmul(dummy[:, 0:128], warm[:, 0:128], warm[:, 128:256],
                         start=True, stop=True, skip_group_check=True)
    # pre-loop: pair 0 q,k in two parallel scratch banks
    p1a_epoch(0, 0, first=True, force=p1p)
    p1a_epoch(1, 0, force=oap)
    pe_mm(dummy[0:1, 0:2], xt[:, 0, 512:514])   # PE absorbs xt chunk 2 wait
    pe_mm(dummy[0:1, 2:4], wvt[:, 0, 0:2])      # PE absorbs wvt DMA wait
    p1a_epoch(0, 1, force=p1p)
    p1a_epoch(1, 1, force=oap)
    dve_sync(bo_sb[0:1, 0:1])                   # bo DMA wait for yA STTs

    # Round plan per loop (pair p scores itself; AV for p-1 lags one loop):
    #  r0-r3: AV(p-1) 2 j's per round    [p==0: p1b + p1a-half alternating]
    #  r4   : norm(p-1)
    #  r5   : transpose(p-1) + resT evict [p==3: AV3 j0,j1]
    #  r6,r7: p1a for p+1                 [p==3: AV3 cont.]
    for pair in range(NPAIR):
        av_pair = pair - 1
        if pair == 1:
            nc.vector.tensor_copy(identb[:, :], ident)
        if pair == 3:
            pending_pe_syncs.append(wot[:, 0, 0:2])
        if av_pair >= 0:
            av_open(av_pair)
        for j in range(NT):
            scores_round(pair, j)
            if pair == 0:
                p1b_epoch(j)
                if j % 2 == 0:
                    p1a_epoch(2 + j // 4, (j % 4) // 2)
            if av_pair >= 0 and j < 4:
                av_round(av_pair, 2 * j)
                av_round(av_pair, 2 * j + 1)
            if pair in (1, 2) and j >= 4:
                p1a_epoch(2 * (pair + 1) + (j - 4) // 2, (j - 4) % 2,
                          force=(p1p if j in (4, 6) else None))
            if pair == 3 and j in (0, 1, 2, 4):
                m = {0: 0, 1: 1, 2: 2, 4: 3}[j]
                ya_group(m, 0)
                ya_group(m, 1)
            if av_pair >= 0 and j == 4:
                resq = norm_pair(av_pair)
                resq_prev = (av_pair, resq)
            if av_pair >= 0 and j == 5:
                tr_pair, tr_resq = resq_prev
                transpose_pair(tr_pair, tr_resq)
            if pair == 3 and j >= 5:
                if j == 5:
                    av_open(NPAIR - 1)
                    av_round(NPAIR - 1, 0)
                    av_round(NPAIR - 1, 1)
                elif j == 6:
                    av_round(NPAIR - 1, 2)
                    av_round(NPAIR - 1, 3)
                    av_round(NPAIR - 1, 4)
                else:
                    av_round(NPAIR - 1, 5)
                    av_round(NPAIR - 1, 6)

    # ---------------- tail ----------------
    av_round(NPAIR - 1, NT - 1)
    resq = norm_pair(NPAIR - 1)
    transpose_pair(NPAIR - 1, resq)

    # yB: remaining P4 contraction (k=2,3) + bias + yax, then y DMA per m
    yr = y.rearrange("(k p) s -> p k s", p=P)
    dve_sync(ets_hist[-1][0:1, 0:16])   # last exp tick (ACT) for the STTs
    for m in range(KT):
        acc = scp.tile([P, S], F32, tag="sc", name=f"ybacc{m}")
        if m == 0:
            pe_mm(acc[0:1, 0:2], ets_hist[-1])
            pe_mm(acc[0:1, 2:4], resT[:, NPAIR - 1, :])
        for n in range(2):
            for k in (2, 3):
                nc.tensor.matmul(
                    acc[:, n * 512:(n + 1) * 512],
                    wot[:, k, m * P:(m + 1) * P],
                    resT[:, k, n * 512:(n + 1) * 512],
                    start=(k == 2), stop=(k == 3),
                )
        dve_sync(acc[0:1, 0:16])
        with nc.allow_low_precision(reason="bf16 y output"):
            nc.vector.scalar_tensor_tensor(
                ybig[:, m, :], acc[:, :], bo_sb[:, m:m + 1],
                yax[:, m, :], op0=ALU.add, op1=ALU.add,
            )
        q = nc.sync if m % 2 == 0 else nc.scalar
        q.dma_start(out=yr[:, m:m + 1, :], in_=ybig[:, m:m + 1, :])


ENGINE_SEM_PREFIX = {
    "PE": "PE_",
    "Activation": "Activation_",
    "Pool": "Pool_",
    "SP": "SP_",
}
# scr is write-only wait-carrier scratch: its DVE self-waits are strippable.
SCRATCH_DVE_PREFIX = {**ENGINE_SEM_PREFIX, "DVE": "DVE_"}


def _strip_self_waits(nc):
    """Drop same-engine semaphore self-waits from multi-wait instructions.

    Engines execute and complete their own instructions in program order,
    so a wait on the engine's own completion semaphore is redundant whenever
    the instruction carries another wait — and walrus's PE/ACT instruction
    structs only encode a single wait.
    """
    n = 0
    for inst in nc.inst_map.values():
        si = getattr(inst, "sync_info", None)
        if si is None or not si.on_wait or len(si.on_wait) <= 1:
            continue
        eng = str(getattr(inst, "engine", "")).split(".")[-1]
        outs = [getattr(a, "memref", "") for a in inst.outs]
        table = (SCRATCH_DVE_PREFIX
                 if outs and all(o.startswith("scr_") for o in outs)
                 else ENGINE_SEM_PREFIX)
        pref = table.get(eng)
        if pref is None:
            continue
        keep = [w for w in si.on_wait if not w.ant_name.startswith(pref)]
        if len(keep) != len(si.on_wait) and keep:
            inst.sync_info = mybir.SyncInfo(
                on_wait=keep, on_update=list(si.on_update or [])
            )
            n += 1
    return n


def build_nc():
    _install_drain_split()
    nc = bass.Bass(trn_type="TRN2", debug=False, num_devices=8)
    x_d = nc.dram_tensor("xb", [C, S], BF16, kind="ExternalInput")
    w_d = nc.dram_tensor("wallb", [C, NWCOL], BF16, kind="ExternalInput")
    b_d = nc.dram_tensor("bof", [C, 1], F32, kind="ExternalInput")
    y_d = nc.dram_tensor("y", [C, S], BF16, kind="ExternalOutput")
    with tile.TileContext(nc) as tc, ExitStack() as ctx:
        trace_kernel(ctx, tc, nc, x_d.ap(), w_d.ap(), b_d.ap(), y_d.ap())
    _strip_self_waits(nc)
    if not nc.is_finalized():
        nc.finalize()
    return nc


def host_inputs(x, Wqkv, Wo, bo):
    """Host-side reshard: per-core input dicts (weights replicated)."""
    x = np.asarray(x, dtype=np.float32)
    Wqkv = np.asarray(Wqkv, dtype=np.float32)
    Wo = np.asarray(Wo, dtype=np.float32)
    bo = np.asarray(bo, dtype=np.float32)

    # Wqkv rows per head h: [h*3D, h*3D+D) = q, [+D, +2D) = k, [+2D, +3D) = v.
    # q,k channel order: per pair -> [q(2p)|q(2p+1)], [k(2p)|k(2p+1)] tiles.
    order = []
    for p in range(NPAIR):
        for h in (2 * p, 2 * p + 1):
            order.extend(range(h * 3 * D, h * 3 * D + D))          # q rows
        for h in (2 * p, 2 * p + 1):
            order.extend(range(h * 3 * D + D, h * 3 * D + 2 * D))  # k rows
    wqkt = Wqkv[order].T                                            # (C, 2C)
    v_order = [h * 3 * D + 2 * D + d for h in range(NH) for d in range(D)]
    wvt = Wqkv[v_order].T                                           # (C, C)
    wot = Wo.T                                                      # (C, C)
    ident = np.zeros((C, P), dtype=np.float32)
    ident[0:P, 0:P] = np.eye(P)
    wallb = np.ascontiguousarray(
        np.concatenate([wqkt, wvt, wot, ident], axis=1)
    ).astype(ml_dtypes.bfloat16)                                    # (C, 4C+128)
    bof = np.ascontiguousarray(bo[:, None])                         # (C, 1)

    xb = x.reshape(B, C, S).astype(ml_dtypes.bfloat16)
    return [
        dict(xb=np.ascontiguousarray(xb[b]), wallb=wallb, bof=bof)
        for b in range(B)
    ]


_NC_CACHE = []

try:
    # bass_exec HLO does not embed the BIR; bust jax's executable cache so a
    # rebuilt kernel is actually recompiled instead of hitting a stale NEFF.
    import jax as _jax

    _jax.clear_caches()
except Exception:
    pass


def get_nc():
    if not _NC_CACHE:
        _NC_CACHE.append(build_nc())
    return _NC_CACHE[0]


def run(in_maps, **kwargs):
    return run_bass_kernel_spmd(get_nc(), in_maps, core_ids=list(range(B)), **kwargs)


def kernel(x, Wqkv, Wo, bo):
    in_maps = host_inputs(x, Wqkv, Wo, bo)
    r = run(in_maps)
    yv = np.stack([r.results[b]["y"].reshape(C, H, W) for b in range(B)])
    return yv.astype(np.float32)


if __name__ == "__main__":
    nc = build_nc()
    print("built ok:", len(nc.inst_map), "instructions")


# revision 33
# speedup vs baseline: 1.0027x; 1.0027x over previous
"""Trainium2 Bass kernel for nn_AttentionBlock (B=8, C=512, H=W=32, 8 heads).

Sharding: data-parallel over batch — core b computes batch image b end-to-end
(weights replicated to all 8 cores).

Design notes (cost model: matmul time = moving-operand columns only; moving
dtype sets the rate, bf16 = 1 col/cycle at any N):
  P1a: q,k = Wqk^T.T @ x -> (1024, S), channel order arranged on host so each
       128-row tile is one head-PAIR of q or k.  All operands bf16.
  P1b: vT = x.T @ Wv^T -> (S, 512) so attention needs no transposes.
  sc : scoresT[key, q] per (j-tile, head): 2 matmuls of N=512 into a PSUM
       ping-pong slot; ACT exp -> et (bf16, SBUF) at scale 1/8.
  AV : FLIPPED vs the usual orientation — out[q-tile, d] = et-chunk^T @ v
       streams only N=64 v-columns per (head, j, q-chunk): 8x less moving
       data than streaming queries.  Denominators via extra N=1 matmuls
       (rhs = ones column) into a dedicated PSUM bank.
       AV for pair p runs one pair-loop LATE (during pair p+1's score/exp
       rounds) so the PE always has exp-independent work while ACT (the
       64x 1038ns exp stream) paces the attention phase.
  norm: DVE reciprocal of the 16 denominator columns, then one stride-0
       broadcast tensor_tensor per head: evict+normalize PSUM->SBUF bf16.
  T  : PE transposes (bf16 identity moving => 1 cycle/row) turn res'[q, c]
       into res[c, q] for the output projection.
  P4 : y = Wo^T.T @ resT + bo + x, STT on DVE, y DMA per m-tile.

PSUM banks: 0-3 scores ping-pong (2 slots x 4KB), 4-5 AV accumulators
(2 heads x 8 q-tiles x 64), 6 P1/transpose scratch, 7 denominators.
Interleaved PSUM accumulation groups share banks via the 2KB zero-region
rule: only the first matmul touching a bank-epoch uses start=True; other
groups' first writes auto-zero through the pending-zero flag.
"""

import os
import sys

for _p in ("/opt/trn_rl_repo", "/root/.axon_site/_ro/trn_rl_repo"):
    if os.path.isdir(_p) and _p not in sys.path:
        sys.path.insert(0, _p)

from contextlib import ExitStack

import ml_dtypes
import numpy as np

import concourse.bass as bass
import concourse.tile as tile
from concourse import mybir
from concourse.bass_utils import run_bass_kernel_spmd

B, C, H, W = 8, 512, 32, 32
NH, D = 8, 64
S = H * W            # 1024 sequence positions
P = 128              # partitions
KT = C // P          # 4 contraction tiles over channels
NT = S // P          # 8 key/query tiles
NPAIR = NH // 2      # 4 head pairs
NWCOL = 4 * C + P    # wallb cols: 2C qk | C v | C wo | 128 identity
F32 = mybir.dt.float32
BF16 = mybir.dt.bfloat16
AF = mybir.ActivationFunctionType
ALU = mybir.AluOpType

EXP_BUFS = int(os.environ.get("K_EXP_BUFS", "24"))
WARM_BIG = int(os.environ.get("K_WARM_BIG", "6"))
WARM_SMALL = int(os.environ.get("K_WARM_SMALL", "2"))


def _install_drain_split():
    """walrus's CTRL_NO (drain) codegen accepts only a single semaphore wait,
    but Tile's kernel-tail drain aggregates one wait per live proc.  Split
    them across several serial drains (semantically identical: all complete
    before the closing all-engine barrier)."""
    if getattr(tile.TileContext, "_drain_split_installed", False):
        return
    from concourse.vector_clock import ScopedClock

    orig = tile.TileContext._drain_and_barrier

    def patched(self, tick_clock, wait_clock):
        nc = self.nc
        drain_inst = nc.sync.drain()
        wait_clock.add_sem_waits(
            drain_inst.ins, ScopedClock({None: tick_clock.global_clock})
        )
        si = drain_inst.ins.sync_info
        if si is not None and si.on_wait and len(si.on_wait) > 1:
            waits = list(si.on_wait)
            drain_inst.ins.sync_info = mybir.SyncInfo(
                on_wait=[waits[0]], on_update=list(si.on_update or [])
            )
            for w in waits[1:]:
                d2 = nc.sync.drain()
                d2.ins.sync_info = mybir.SyncInfo(on_wait=[w], on_update=[])

        nc.all_engine_barrier()
        assert self.sems is not None
        popped = nc._tile_sem_poison_stack.pop()
        assert popped is self._sem_poison
        nc.clear_and_free_semaphores(list(self.sems.allocated().values()))
        nc.all_engine_barrier()

    tile.TileContext._drain_and_barrier = patched
    tile.TileContext._drain_split_installed = True
    tile.TileContext._drain_and_barrier_orig = orig


def trace_kernel(ctx, tc, nc, xb, wallb, bof, y):
    cst = ctx.enter_context(tc.tile_pool(name="cst", bufs=1))
    qkp = ctx.enter_context(tc.tile_pool(name="qkp", bufs=4))
    expp = ctx.enter_context(tc.tile_pool(name="expp", bufs=EXP_BUFS))
    rdp = ctx.enter_context(tc.tile_pool(name="rdp", bufs=2))
    rqp = ctx.enter_context(tc.tile_pool(name="rqp", bufs=2))
    yp = ctx.enter_context(tc.tile_pool(name="yp", bufs=1))
    # PSUM pools, allocation order = bank order (8 banks total):
    # scp: scores ping-pong 2x[128,1024] (4 banks) + yB accs at the tail;
    # oap: AV accumulators / spare P1 scratch (2 banks); p1p: P1/transpose/yA
    # scratch (1 bank); dnp: denominators (1 bank).
    scp = ctx.enter_context(tc.tile_pool(name="scp", bufs=2, space="PSUM"))
    oap = ctx.enter_context(tc.tile_pool(name="oap", bufs=1, space="PSUM"))
    p1p = ctx.enter_context(tc.tile_pool(name="p1p", bufs=1, space="PSUM"))
    dnp = ctx.enter_context(tc.tile_pool(name="dnp", bufs=1, space="PSUM"))

    xt = cst.tile([P, KT, S], BF16)
    wall = cst.tile([P, KT, NWCOL], BF16)
    wqkt = wall[:, :, 0:2 * C]
    wvt = wall[:, :, 2 * C:3 * C]
    wot = wall[:, :, 3 * C:4 * C]
    ident = wall[:, 0, 4 * C:4 * C + P]          # [128, 128] bf16 identity

    bo_sb = cst.tile([P, KT], F32)
    onesc = cst.tile([P, 1], BF16)
    vta = cst.tile([P, NT, C], BF16)             # v^T tiles, head-major cols
    resT = cst.tile([P, KT, S], BF16)            # res[c, s], k-tile = pair
    scr = cst.tile([1, 256], F32)
    scrp = cst.tile([1, 16], F32)    # Pool-engine carrier scratch
    identb = cst.tile([P, P], BF16)
    warm = cst.tile([P, 640], BF16)
    yax = cst.tile([P, KT, S], F32)              # yA partial (k=0,1) + x
    ybig = yp.tile([P, KT, S], BF16)

    # ---- input DMA, chunked so the first p1a epoch starts early ----
    xr = xb.rearrange("(k p) s -> p k s", p=P)
    wr = wallb.rearrange("(k p) c -> p k c", p=P)
    nc.gpsimd.dma_start(out=wall[:, :, 0:256], in_=wr[:, :, 0:256])
    nc.sync.dma_start(out=xt[:, :, 0:512], in_=xr[:, :, 0:512])
    nc.sync.dma_start(out=xt[:, :, 512:S], in_=xr[:, :, 512:S])
    nc.gpsimd.dma_start(out=wall[:, :, 2 * C:3 * C], in_=wr[:, :, 2 * C:3 * C])
    nc.gpsimd.dma_start(out=wall[:, :, 256:2 * C], in_=wr[:, :, 256:2 * C])
    nc.gpsimd.dma_start(out=wall[:, :, 3 * C:NWCOL], in_=wr[:, :, 3 * C:NWCOL])
    nc.gpsimd.dma_start(out=bo_sb.unsqueeze(2),
                        in_=bof.rearrange("(k p) o -> p k o", p=P))

    nc.vector.memset(onesc[:, :], 1.0)

    scr_i = [0]

    def dve_sync(*aps):
        # DVE wait-carrier: absorb one cross-engine wait per tiny copy.
        # Callers pass 2-D APs (partition x free).
        for ap in aps:
            n = min(ap.free_size(), 16)
            o = (scr_i[0] % 15) * 16
            scr_i[0] += 1
            nc.vector.tensor_copy(scr[0:1, o:o + n], ap[0:1, 0:n])

    def pe_mm(corner, dep):
        # PE wait-carrier: a 1x2 matmul reading `dep` absorbs one cross-
        # engine wait; PE program order subsumes the tick for later matmuls.
        nc.tensor.matmul(
            corner, dep[0:1, 0:1], dep[0:1, 0:2],
            start=True, stop=True, skip_group_check=True,
        )

    # ---------------- scores + exp (2-slot ping-pong, 2 exps/j) ------------
    pending_pe_syncs = []
    ets_hist = []
    pair_ets = [[None] * (2 * NT) for _ in range(NPAIR)]

    def scores_round(pair, j):
        qk = qk_tiles[pair]
        for hh in range(2):
            sc = scp.tile([P, S], F32, tag="sc", name=f"sc{pair}_{j}_{hh}")
            idx = len(ets_hist)
            if idx >= 2:
                pe_mm(sc[0:1, 0:2], ets_hist[idx - 2])
            while pending_pe_syncs:
                pe_mm(sc[0:1, 0:2], pending_pe_syncs.pop())
            for n in range(2):
                nc.tensor.matmul(
                    sc[:, n * 512:(n + 1) * 512],
                    qk[64 * hh:64 * (hh + 1), S + j * P: S + (j + 1) * P],
                    qk[64 * hh:64 * (hh + 1), n * 512:(n + 1) * 512],
                    start=True, stop=True,
                )
            et = expp.tile([P, S], BF16, tag="et", name=f"et{pair}_{j}_{hh}")
            nc.scalar.activation(et[:, :], sc[:, :], AF.Exp,
                                 scale=1.0 / np.sqrt(D))
            ets_hist.append(et)
            pair_ets[pair][2 * j + hh] = et

    # ---------------- P1 epochs on alternating scratch banks ---------------
    qk_tiles = [None] * NPAIR
    use_oap = [False]

    def scratch(name, force=None):
        # alternate between the p1p bank and the (free) oap bank
        if force is None:
            use_oap[0] = not use_oap[0]
            pool = oap if use_oap[0] else p1p
        else:
            pool = force
        return pool.tile([P, 512], F32, tag="oa" if pool is oap else "p1",
                         name=name)

    def p1a_epoch(m, n, first=False, force=None):
        pair, isk = divmod(m, 2)
        if isk == 0 and n == 0:
            qk_tiles[pair] = qkp.tile([P, 2 * S], BF16, tag="qk",
                                      name=f"qk{pair}")
        acc = scratch(f"p1a{m}_{n}", force)
        if first:
            pe_mm(acc[0:1, 0:2], xt[:, 0, 0:2])
            pe_mm(acc[0:1, 0:2], wall[:, 0, 0:2])
        for k in range(KT):
            nc.tensor.matmul(
                acc[:, :],
                wqkt[:, k, m * P:(m + 1) * P],
                xt[:, k, n * 512:(n + 1) * 512],
                start=(k == 0), stop=(k == KT - 1),
            )
        with nc.allow_low_precision(reason="bf16 qk tiles"):
            nc.vector.tensor_copy(
                qk_tiles[pair][:, isk * S + n * 512: isk * S + (n + 1) * 512],
                acc[:, :],
            )

    def p1b_epoch(j):
        acc = scratch(f"p1b{j}")
        for k in range(KT):
            nc.tensor.matmul(
                acc[:, :],
                xt[:, k, j * P:(j + 1) * P],
                wvt[:, k, :],
                start=(k == 0), stop=(k == KT - 1),
            )
        with nc.allow_low_precision(reason="bf16 v tiles"):
            nc.vector.tensor_copy(vta[:, j, :], acc[:, :])

    def ya_group(m, n):
        acc = scratch(f"ya{m}_{n}", force=p1p)
        for k in range(2):
            nc.tensor.matmul(
                acc[:, :],
                wot[:, k, m * P:(m + 1) * P],
                resT[:, k, n * 512:(n + 1) * 512],
                start=(k == 0), stop=(k == 1),
            )
        nc.vector.scalar_tensor_tensor(
            yax[:, m, n * 512:(n + 1) * 512], acc[:, :], 0.0,
            xt[:, m, n * 512:(n + 1) * 512], op0=ALU.add, op1=ALU.add,
        )

    # ---------------- AV + norm + transpose --------------------------------
    av_state = {}

    def av_open(pair):
        av_state["oa"] = oap.tile([P, 1024], F32, tag="oa", name=f"oa{pair}")
        av_state["den"] = dnp.tile([P, 16], F32, tag="den", name=f"den{pair}")

    def av_round(pair, j):
        oa, den = av_state["oa"], av_state["den"]
        for hh in range(2):
            et = pair_ets[pair][2 * j + hh]
            v = vta[:, j, (2 * pair + hh) * D:(2 * pair + hh + 1) * D]
            for t in range(NT):
                nc.tensor.matmul(
                    oa[:, hh * 512 + t * D: hh * 512 + (t + 1) * D],
                    et[:, t * P:(t + 1) * P], v,
                    start=(j == 0 and t == 0), stop=(j == NT - 1),
                    skip_group_check=True,
                )
                nc.tensor.matmul(
                    den[:, hh * NT + t: hh * NT + t + 1],
                    et[:, t * P:(t + 1) * P], onesc[:, :],
                    start=(j == 0 and t == 0 and hh == 0),
                    stop=(j == NT - 1),
                    skip_group_check=True,
                )

    def norm_pair(pair):
        oa, den = av_state["oa"], av_state["den"]
        # sample-read carriers absorb the PE waits (schedule-correct values)
        dve_sync(den[0:1, 0:16])
        for hh in range(2):
            dve_sync(oa[0:1, hh * 512:(hh + 1) * 512].rearrange(
                "p (t d) -> p t d", t=NT)[:, :, 0:1].rearrange(
                "p t d -> p (t d)"))
        rd = rdp.tile([P, 16], F32, tag="rd", name=f"rd{pair}")
        nc.vector.reciprocal(rd[:, :], den[:, :])
        resq = rqp.tile([P, NT, P], BF16, tag="rq", name=f"resq{pair}")
        with nc.allow_low_precision(reason="bf16 res tiles"):
            for hh in range(2):
                nc.vector.tensor_tensor(
                    resq[:, :, hh * D:(hh + 1) * D],
                    oa[:, hh * 512:(hh + 1) * 512].rearrange(
                        "p (t d) -> p t d", t=NT),
                    rd[:, hh * NT:(hh + 1) * NT].unsqueeze(2).broadcast_to(
                        [P, NT, D]),
                    op=ALU.mult,
                )
        return resq

    def transpose_pair(pair, resq):
        tp = p1p.tile([P, NT * P], BF16, tag="p1", name=f"tp{pair}")
        for t in range(NT):
            nc.tensor.transpose(
                tp[:, t * P:(t + 1) * P], resq[:, t, :], identb[:, :])
        nc.vector.tensor_copy(resT[:, pair, :], tp[:, :])

    # ================= schedule =================
    dummy = scp.tile([P, S], F32, tag="sc", name="dummy")
    dve_sync(xt[0:1, 0, 0:8])
    dve_sync(xt[0:1, 0, 512:520])
    # warm-up: spin the PE while the input DMA lands so the p-state ramp
    # completes before real work
    nc.vector.memset(warm[:, :], 0.25)
    for i in range(WARM_BIG):
        nc.tensor.matmul(dummy[:, 0:512], warm[:, 0:128], warm[:, 128:640],
                         start=True, stop=True, skip_group_check=True)
    for i in range(WARM_SMALL):
        nc.tensor.matmul(dummy[:, 0:128], warm[:, 0:128], warm[:, 128:256],
                         start=True, stop=True, skip_group_check=True)
    # pre-loop: pair 0 q,k in two parallel scratch banks
    p1a_epoch(0, 0, first=True, force=p1p)
    p1a_epoch(1, 0, force=oap)
    pe_mm(dummy[0:1, 0:2], xt[:, 0, 512:514])   # PE absorbs xt chunk 2 wait
    p1a_epoch(0, 1, force=p1p)
    p1a_epoch(1, 1, force=oap)
    pending_pe_syncs.append(wvt[:, 0, 0:2])     # wvt DMA wait, for p1b
    dve_sync(bo_sb[0:1, 0:1])                   # bo DMA wait for yA STTs

    # Round plan per loop (pair p scores itself; AV for p-1 lags one loop):
    #  r0-r3: AV(p-1) 2 j's per round    [p==0: p1b + p1a-half alternating]
    #  r4   : norm(p-1)
    #  r5   : transpose(p-1) + resT evict [p==3: AV3 j0,j1]
    #  r6,r7: p1a for p+1                 [p==3: AV3 cont.]
    for pair in range(NPAIR):
        av_pair = pair - 1
        if pair == 1:
            nc.vector.tensor_copy(identb[:, :], ident)
        if pair == 3:
            pending_pe_syncs.append(wot[:, 0, 0:2])
        if av_pair >= 0:
            av_open(av_pair)
        for j in range(NT):
            scores_round(pair, j)
            if pair == 0:
                if j == 1:
                    p1b_epoch(0)
                if j >= 1:
                    p1b_epoch(j)
                if j in (2, 3, 5, 6):
                    i = (2, 3, 5, 6).index(j)
                    p1a_epoch(2 + i // 2, i % 2)
            if av_pair >= 0 and j < 4:
                av_round(av_pair, 2 * j)
                av_round(av_pair, 2 * j + 1)
            if pair in (1, 2) and j >= 4:
                p1a_epoch(2 * (pair + 1) + (j - 4) // 2, (j - 4) % 2,
                          force=(p1p if j in (4, 6) else None))
            if pair == 3 and j in (0, 1, 2, 4):
                m = {0: 0, 1: 1, 2: 2, 4: 3}[j]
                ya_group(m, 0)
                ya_group(m, 1)
            if av_pair >= 0 and j == 4:
                resq = norm_pair(av_pair)
                resq_prev = (av_pair, resq)
            if av_pair >= 0 and j == 5:
                tr_pair, tr_resq = resq_prev
                transpose_pair(tr_pair, tr_resq)
            if pair == 3 and j >= 5:
                if j == 5:
                    av_open(NPAIR - 1)
                    av_round(NPAIR - 1, 0)
                    av_round(NPAIR - 1, 1)
                elif j == 6:
                    av_round(NPAIR - 1, 2)
                    av_round(NPAIR - 1, 3)
                    av_round(NPAIR - 1, 4)
                else:
                    av_round(NPAIR - 1, 5)
                    av_round(NPAIR - 1, 6)

    # ---------------- tail ----------------
    av_round(NPAIR - 1, NT - 1)
    resq = norm_pair(NPAIR - 1)
    transpose_pair(NPAIR - 1, resq)

    # yB: remaining P4 contraction (k=2,3) + bias + yax, then y DMA per m
    yr = y.rearrange("(k p) s -> p k s", p=P)
    dve_sync(ets_hist[-1][0:1, 0:16])   # last exp tick (ACT) for the STTs
    for m in range(KT):
        acc = scp.tile([P, S], F32, tag="sc", name=f"ybacc{m}")
        if m == 0:
            pe_mm(acc[0:1, 0:2], ets_hist[-1])
            pe_mm(acc[0:1, 2:4], resT[:, NPAIR - 1, :])
        for n in range(2):
            for k in (2, 3):
                nc.tensor.matmul(
                    acc[:, n * 512:(n + 1) * 512],
                    wot[:, k, m * P:(m + 1) * P],
                    resT[:, k, n * 512:(n + 1) * 512],
                    start=(k == 2), stop=(k == 3),
                )
        dve_sync(acc[0:1, 0:16])
        with nc.allow_low_precision(reason="bf16 y output"):
            nc.vector.scalar_tensor_tensor(
                ybig[:, m, :], acc[:, :], bo_sb[:, m:m + 1],
                yax[:, m, :], op0=ALU.add, op1=ALU.add,
            )
        q = nc.sync if m % 2 == 0 else nc.scalar
        q.dma_start(out=yr[:, m:m + 1, :], in_=ybig[:, m:m + 1, :])


ENGINE_SEM_PREFIX = {
    "PE": "PE_",
    "Activation": "Activation_",
    "Pool": "Pool_",
    "SP": "SP_",
}
# scr is write-only wait-carrier scratch: its DVE self-waits are strippable.
SCRATCH_DVE_PREFIX = {**ENGINE_SEM_PREFIX, "DVE": "DVE_"}


def _strip_self_waits(nc):
    """Drop same-engine semaphore self-waits from multi-wait instructions.

    Engines execute and complete their own instructions in program order,
    so a wait on the engine's own completion semaphore is redundant whenever
    the instruction carries another wait — and walrus's PE/ACT instruction
    structs only encode a single wait.
    """
    n = 0
    for inst in nc.inst_map.values():
        si = getattr(inst, "sync_info", None)
        if si is None or not si.on_wait or len(si.on_wait) <= 1:
            continue
        eng = str(getattr(inst, "engine", "")).split(".")[-1]
        outs = [getattr(a, "memref", "") for a in inst.outs]
        table = (SCRATCH_DVE_PREFIX
                 if outs and all(o.startswith("scr_") for o in outs)
                 else ENGINE_SEM_PREFIX)
        pref = table.get(eng)
        if pref is None:
            continue
        keep = [w for w in si.on_wait if not w.ant_name.startswith(pref)]
        if len(keep) != len(si.on_wait) and keep:
            inst.sync_info = mybir.SyncInfo(
                on_wait=keep, on_update=list(si.on_update or [])
            )
            n += 1
    return n


def build_nc():
    _install_drain_split()
    nc = bass.Bass(trn_type="TRN2", debug=False, num_devices=8)
    x_d = nc.dram_tensor("xb", [C, S], BF16, kind="ExternalInput")
    w_d = nc.dram_tensor("wallb", [C, NWCOL], BF16, kind="ExternalInput")
    b_d = nc.dram_tensor("bof", [C, 1], F32, kind="ExternalInput")
    y_d = nc.dram_tensor("y", [C, S], BF16, kind="ExternalOutput")
    with tile.TileContext(nc) as tc, ExitStack() as ctx:
        trace_kernel(ctx, tc, nc, x_d.ap(), w_d.ap(), b_d.ap(), y_d.ap())
    _strip_self_waits(nc)
    if not nc.is_finalized():
        nc.finalize()
    return nc


def host_inputs(x, Wqkv, Wo, bo):
    """Host-side reshard: per-core input dicts (weights replicated)."""
    x = np.asarray(x, dtype=np.float32)
    Wqkv = np.asarray(Wqkv, dtype=np.float32)
    Wo = np.asarray(Wo, dtype=np.float32)
    bo = np.asarray(bo, dtype=np.float32)

    # Wqkv rows per head h: [h*3D, h*3D+D) = q, [+D, +2D) = k, [+2D, +3D) = v.
    # q,k channel order: per pair -> [q(2p)|q(2p+1)], [k(2p)|k(2p+1)] tiles.
    order = []
    for p in range(NPAIR):
        for h in (2 * p, 2 * p + 1):
            order.extend(range(h * 3 * D, h * 3 * D + D))          # q rows
        for h in (2 * p, 2 * p + 1):
            order.extend(range(h * 3 * D + D, h * 3 * D + 2 * D))  # k rows
    wqkt = Wqkv[order].T                                            # (C, 2C)
    v_order = [h * 3 * D + 2 * D + d for h in range(NH) for d in range(D)]
    wvt = Wqkv[v_order].T                                           # (C, C)
    wot = Wo.T                                                      # (C, C)
    ident = np.zeros((C, P), dtype=np.float32)
    ident[0:P, 0:P] = np.eye(P)
    wallb = np.ascontiguousarray(
        np.concatenate([wqkt, wvt, wot, ident], axis=1)
    ).astype(ml_dtypes.bfloat16)                                    # (C, 4C+128)
    bof = np.ascontiguousarray(bo[:, None])                         # (C, 1)

    xb = x.reshape(B, C, S).astype(ml_dtypes.bfloat16)
    return [
        dict(xb=np.ascontiguousarray(xb[b]), wallb=wallb, bof=bof)
        for b in range(B)
    ]


_NC_CACHE = []

try:
    # bass_exec HLO does not embed the BIR; bust jax's executable cache so a
    # rebuilt kernel is actually recompiled instead of hitting a stale NEFF.
    import jax as _jax

    _jax.clear_caches()
except Exception:
    pass


def get_nc():
    if not _NC_CACHE:
        _NC_CACHE.append(build_nc())
    return _NC_CACHE[0]


def run(in_maps, **kwargs):
    return run_bass_kernel_spmd(get_nc(), in_maps, core_ids=list(range(B)), **kwargs)


def kernel(x, Wqkv, Wo, bo):
    in_maps = host_inputs(x, Wqkv, Wo, bo)
    r = run(in_maps)
    yv = np.stack([r.results[b]["y"].reshape(C, H, W) for b in range(B)])
    return yv.astype(np.float32)


if __name__ == "__main__":
    nc = build_nc()
    print("built ok:", len(nc.inst_map), "instructions")
